# revision 12
# baseline (speedup 1.0000x reference)
"""Masked dot-product attention (B=64, L=1024, D=64, fp32) on 8 NeuronCores.

Strategy (data-parallel over batch, per the sharding hint):
  - Batches are sorted by valid_len (descending) and dealt round-robin to the
    8 cores; slot s's key-block loop count is the max over its rank group, so
    one SPMD instruction stream fits all cores and fully-masked key blocks
    are never computed.
  - The sequence mask rides INSIDE the QK matmul as an extra contraction row:
    K^T is augmented with a mask row (0 / -60000 per key) and Q^T with a ones
    row, so S' = K^T.T @ Q^T + m[k] needs no per-block exp bias.  That makes
    the exp a pure elementwise op over PSUM columns, so one ScalarE
    instruction spans THREE 512-column score units regardless of which key
    block they belong to (amortizes the ~185ns per-instruction SBUF access
    latency; ScalarE is the bottleneck engine at ~1 elem/cycle/partition).
  - Scores are computed transposed, S^T[k, q], 512 q at a time:
    matmul(lhsT=K^T_aug[65, 128], rhs=Q^T_aug[65, 512]) -> PSUM [128, 512].
    Work is a flat stream of (slot, kb, qh) units; exp groups of 3 units
    live in [128, 1536] PSUM tiles (3 banks, double-buffered = 6 banks).
  - P = exp(S'/8) is written as float16 (rel err ~5e-4, well inside the
    2e-2 gate).  AV is P-chunk-stationary: for each 128-query block,
    matmul(lhsT=P^T[128k, 128q], rhs=V_aug[128k, 65]) accumulates
    O[q, d] over key blocks in PSUM.  LdWeights is free on the PE, and the
    65 fp16 moving rows cost 65 cycles, so AV is ~2x cheaper than the
    moving-P orientation and the output lands Q-MAJOR.
  - V_aug has a ones column, so O[:, 64] is the softmax denominator.
    Normalization collapses to a [128,1]-per-partition scalar multiply:
    one DVE reciprocal on the 4 denominator columns + one tensor_tensor
    with a stride-0 broadcast AP.  No cross-partition broadcast of any
    kind (the baseline's PE/DMA reciprocal-row machinery is gone).
  - Outputs are written [q, d] per slot — the natural layout — so the
    host-side unshard is a pure batch reorder.

Engine budget per core (cost model): ScalarE ~42us (saturated), PE ~28us,
DVE ~8us, Pool: v-loads only.  Inputs fp16 (Q/K host-converted; scores err
~4e-3 absolute pre-softmax-scale), matmul accumulation in PSUM f32.
"""

import math
from contextlib import ExitStack

import numpy as np

import concourse.bass as bass
import concourse.bacc as bacc
import concourse.mybir as mybir
import concourse.tile as tile
from concourse.bass_utils import run_bass_kernel_spmd

F32 = mybir.dt.float32
F16 = mybir.dt.float16
EXP = mybir.ActivationFunctionType.Exp

B, L, D = 64, 1024, 64
N_CORES = 8
SLOTS = B // N_CORES  # batches per core
KB = 128              # key-block size (partition dim of S^T)
N_KB = L // KB
QH = 512              # q columns per matmul unit (moving-operand max)
NQH = L // QH
GROUP = 3             # 512-col score units per exp instruction (3 PSUM banks)
MASK_VAL = -60000.0   # fits fp16; exp(-60000/8) == 0
DV = D + 1            # V columns + ones (denominator) column


def build_kernel(counts):
    """counts[s] = number of 128-wide key blocks to process for slot s."""
    nc = bacc.Bacc()

    qt_d = nc.dram_tensor("qt", [SLOTS, DV, L], F16, kind="ExternalInput")
    kt_d = nc.dram_tensor("kt", [SLOTS, DV, L], F16, kind="ExternalInput")
    v_d = nc.dram_tensor("v", [SLOTS, KB, N_KB, DV], F16, kind="ExternalInput")
    out_d = nc.dram_tensor("out", [SLOTS, L, D], F32, kind="ExternalOutput")

    # Flat unit stream, qh-major inside each slot so the first exp only
    # needs half of qt; kb ascending keeps PSUM accumulation ordered.
    # Each unit carries its index within the slot (prefetch trigger points).
    units = []
    for s in range(SLOTS):
        for qh in range(NQH):
            for kb in range(counts[s]):
                units.append((s, kb, qh, qh * counts[s] + kb))
    # First groups are small so ScalarE starts as early as possible.
    lead = [1, 1, 2]
    groups = []
    pos = 0
    for n in lead:
        if pos < len(units):
            groups.append(units[pos : pos + n])
            pos += n
    while pos < len(units):
        groups.append(units[pos : pos + GROUP])
        pos += GROUP
    n_groups = len(groups)

    with tile.TileContext(nc) as tc, ExitStack() as ctx:
        const_pool = ctx.enter_context(tc.tile_pool(name="const", bufs=1))
        qk_pool = ctx.enter_context(tc.tile_pool(name="qk", bufs=3))
        v_pool = ctx.enter_context(tc.tile_pool(name="v", bufs=3))
        p_pool = ctx.enter_context(tc.tile_pool(name="p", bufs=4))
        ep_pool = ctx.enter_context(tc.tile_pool(name="ep", bufs=4))
        out_pool = ctx.enter_context(tc.tile_pool(name="out", bufs=4))
        psum_s = ctx.enter_context(tc.tile_pool(name="psum_s", bufs=2, space="PSUM"))
        psum_o = ctx.enter_context(tc.tile_pool(name="psum_o", bufs=1, space="PSUM"))

        qt_tiles: dict[int, object] = {}
        kt_tiles: dict[int, object] = {}
        v_tiles: dict[int, object] = {}
        o_tiles: dict[tuple, object] = {}
        s_tiles: dict[int, object] = {}
        p_tiles: dict[int, object] = {}

        def load_kt(s):
            if s >= SLOTS or s in kt_tiles:
                return
            n_kb = counts[s]
            kt_t = qk_pool.tile([DV, L], F16, tag="kt", name="kt_t")
            if s == 0:
                # Piecewise so the first QK only waits on 256B/partition.
                nc.sync.dma_start(kt_t[:, :KB], kt_d[0][:, :KB])
                if n_kb > 1:
                    nc.sync.dma_start(
                        kt_t[:, KB : n_kb * KB], kt_d[0][:, KB : n_kb * KB]
                    )
            else:
                nc.sync.dma_start(kt_t[:, : n_kb * KB], kt_d[s][:, : n_kb * KB])
            kt_tiles[s] = kt_t

        def load_qt(s):
            if s >= SLOTS or s in qt_tiles:
                return
            qt_t = qk_pool.tile([DV, L], F16, tag="qt", name="qt_t")
            if s == 0:
                # First piece covers the split first QK; the [512:] tail is
                # deferred to group 2 so its DGE config on the Activation
                # sequencer never sits in front of the first exps.
                nc.scalar.dma_start(qt_t[:, : QH // 2], qt_d[0][:, : QH // 2])
                nc.scalar.dma_start(qt_t[:, QH // 2 : QH], qt_d[0][:, QH // 2 : QH])
            else:
                nc.scalar.dma_start(qt_t[:], qt_d[s])
            qt_tiles[s] = qt_t

        def load_v(s):
            if s >= SLOTS or s in v_tiles:
                return
            n_kb = counts[s]
            v_t = v_pool.tile([KB, N_KB, DV], F16, name="v_t")
            nc.gpsimd.dma_start(v_t[:, :n_kb, :], v_d[s][:, :n_kb, :])
            v_tiles[s] = v_t

        def emit_qk(i):
            st = psum_s.tile([KB, GROUP * QH], F32, name="s_ps")
            s_tiles[i] = st
            for u, (s, kb, qh, islot) in enumerate(groups[i]):
                if islot == 0:
                    load_kt(s + 1)
                    load_v(s + 1)
                elif islot == 1:
                    load_qt(s + 1)
                kt_ap = kt_tiles[s][:, kb * KB : (kb + 1) * KB]
                if i == 0:
                    # Split the very first matmul so the first exp can start
                    # as soon as 256 q columns of scores exist.  Both halves
                    # live in one PSUM bank, and start=True zeroes the whole
                    # bank — only the first half may use it.
                    for h in range(2):
                        c0, c1 = h * QH // 2, (h + 1) * QH // 2
                        nc.tensor.matmul(
                            st[:, c0:c1], kt_ap, qt_tiles[s][:, c0:c1],
                            start=(h == 0), stop=True, skip_group_check=True,
                        )
                else:
                    nc.tensor.matmul(
                        st[:, u * QH : (u + 1) * QH],
                        kt_ap,
                        qt_tiles[s][:, qh * QH : (qh + 1) * QH],
                        start=True,
                        stop=True,
                    )

        def emit_exp(i):
            w = len(groups[i]) * QH
            pt = p_pool.tile([KB, GROUP * QH], F16, name="p_t")
            p_tiles[i] = pt
            st = s_tiles.pop(i)
            ranges = [(0, QH // 2), (QH // 2, QH)] if i == 0 else [(0, w)]
            for c0, c1 in ranges:
                nc.scalar.activation(
                    pt[:, c0:c1], st[:, c0:c1], EXP, scale=1.0 / math.sqrt(D)
                )

        def emit_av(i):
            pt = p_tiles.pop(i)
            for u, (s, kb, qh, islot) in enumerate(groups[i]):
                tag = "oA" if qh == 0 else "oB"
                if kb == 0:
                    o_tiles[(s, qh)] = psum_o.tile(
                        [KB, NQH * 2 * DV], F32, tag=tag, name=tag
                    )
                o = o_tiles[(s, qh)]
                last = kb == counts[s] - 1
                for j in range(4):
                    # start=True zeroes the WHOLE PSUM bank, so only the
                    # very first matmul into this o tile may use it; the
                    # other three q-block regions accumulate onto the
                    # zeroed bank.
                    nc.tensor.matmul(
                        o[:, j * DV : (j + 1) * DV],
                        pt[:, u * QH + j * KB : u * QH + (j + 1) * KB],
                        v_tiles[s][:, kb, :],
                        start=(kb == 0 and j == 0),
                        stop=last,
                        skip_group_check=True,
                    )
                if last:
                    emit_epilogue(s, qh)
                    if qh == NQH - 1:
                        qt_tiles.pop(s)
                        kt_tiles.pop(s)
                        v_tiles.pop(s)

        def emit_epilogue(s, half):
            o = o_tiles.pop((s, half))
            rec = ep_pool.tile([KB, 4], F32, name="rec")
            nc.vector.reciprocal(rec[:], o[:, D::DV])
            o3 = o[:].rearrange("p (a b) -> p a b", b=DV)[:, :, :D]
            rec3 = rec[:].rearrange("p (a b) -> p a b", b=1)
            final = s == SLOTS - 1 and half == NQH - 1
            osb = out_pool.tile([KB, 4 * D], F32, name="osb")
            nc.vector.tensor_tensor(
                osb[:].rearrange("p (a b) -> p a b", b=D),
                o3,
                rec3.broadcast_to([KB, 4, D]),
                op=mybir.AluOpType.mult,
            )
            dst = out_d[s][half * QH : (half + 1) * QH].rearrange(
                "(j p) d -> p j d", p=KB
            )
            # Non-final halves ride SWDGE so the SP queue stays clear of
            # head-of-line blocking; the kernel-ending DMA takes the
            # shorter HWDGE fixed-latency chain.
            dma = nc.sync if final else nc.gpsimd
            dma.dma_start(dst, osb[:].rearrange("p (a b) -> p a b", b=D))

        # Prologue: warm the exp table off the critical path, start loads.
        warm_in = const_pool.tile([1, 1], F32)
        warm_out = const_pool.tile([1, 1], F32)
        nc.gpsimd.memset(warm_in[:], 0.0)
        nc.scalar.activation(warm_out[:], warm_in[:], EXP)
        load_kt(0)
        load_qt(0)
        load_v(0)

        emit_qk(0)
        for i in range(n_groups):
            if i == 2:
                # Deferred tail of slot 0's qt (needed from its qh=1 sweep).
                nc.scalar.dma_start(qt_tiles[0][:, QH:], qt_d[0][:, QH:])
            if i + 1 < n_groups:
                emit_qk(i + 1)
            emit_exp(i)
            if i >= 1:
                emit_av(i - 1)
        emit_av(n_groups - 1)

    nc.finalize()
    return nc


_NC_CACHE: dict[tuple, object] = {}


def _prepare(queries, keys, values, valid_lens):
    queries = np.ascontiguousarray(queries, dtype=np.float32)
    keys = np.ascontiguousarray(keys, dtype=np.float32)
    values = np.ascontiguousarray(values, dtype=np.float32)
    valid_lens = np.asarray(valid_lens)
    assert queries.shape == (B, L, D), queries.shape
    vl = valid_lens.astype(np.int64)

    # Sort batches by valid_len descending; slot s on core c gets the batch
    # of rank s*8 + c.  counts[s] covers the rank-group max, so one SPMD
    # instruction stream fits all cores.
    order = np.argsort(-vl, kind="stable")
    counts = tuple(
        max(1, math.ceil(int(vl[order[s * N_CORES]]) / KB)) for s in range(SLOTS)
    )
    nc = _NC_CACHE.get(counts)
    if nc is None:
        nc = build_kernel(counts)
        _NC_CACHE[counts] = nc

    col = np.arange(L)
    in_maps = []
    for c in range(N_CORES):
        batch_idx = [int(order[s * N_CORES + c]) for s in range(SLOTS)]
        # Q^T / K^T with the extra contraction row: ones for Q, mask for K.
        qt = np.empty((SLOTS, DV, L), np.float16)
        qt[:, :D, :] = queries[batch_idx].transpose(0, 2, 1)
        qt[:, D, :] = 1.0
        kt = np.empty((SLOTS, DV, L), np.float16)
        kt[:, :D, :] = keys[batch_idx].transpose(0, 2, 1)
        kt[:, D, :] = (col[None, :] >= vl[batch_idx, None]) * np.float16(MASK_VAL)
        # V with ones column, pre-tiled [KB, N_KB, DV] per slot.
        v = np.empty((SLOTS, L, DV), np.float16)
        v[:, :, :D] = values[batch_idx]
        v[:, :, D] = 1.0
        v = np.ascontiguousarray(
            v.reshape(SLOTS, N_KB, KB, DV).transpose(0, 2, 1, 3)
        )
        in_maps.append({"qt": qt, "kt": kt, "v": v})
    return nc, in_maps, order


def _unshard(res, order):
    out = np.empty((B, L, D), dtype=np.float32)
    for c in range(N_CORES):
        o = res.results[c]["out"]  # [SLOTS, L, D]
        for s in range(SLOTS):
            out[int(order[s * N_CORES + c])] = o[s]
    return out


def kernel(queries, keys, values, valid_lens):
    nc, in_maps, order = _prepare(queries, keys, values, valid_lens)
    res = run_bass_kernel_spmd(nc, in_maps, core_ids=list(range(N_CORES)))
    return _unshard(res, order)


def trace_run(queries, keys, values, valid_lens):
    """Like kernel() but traced; returns BassKernelResults (for test.py)."""
    nc, in_maps, order = _prepare(queries, keys, values, valid_lens)
    res = run_bass_kernel_spmd(
        nc, in_maps, core_ids=list(range(N_CORES)), trace=True
    )
    res.full_output = _unshard(res, order)
    return res


# revision 18
# speedup vs baseline: 1.0606x; 1.0606x over previous
"""Masked dot-product attention (B=64, L=1024, D=64, fp32) on 8 NeuronCores.

Strategy (data-parallel over batch, per the sharding hint):
  - Batches are sorted by valid_len (descending) and dealt round-robin to the
    8 cores; slot s's key-block loop count is the max over its rank group, so
    one SPMD instruction stream fits all cores and fully-masked key blocks
    are never computed.
  - The sequence mask rides INSIDE the QK matmul as an extra contraction row:
    K^T is augmented with a mask row (0 / -60000 per key) and Q^T with a ones
    row, so S' = K^T.T @ Q^T + m[k] needs no per-block exp bias.  That makes
    the exp a pure elementwise op over PSUM columns, so one ScalarE
    instruction spans THREE 512-column score units regardless of which key
    block they belong to (amortizes the ~185ns per-instruction SBUF access
    latency; ScalarE is the bottleneck engine at ~1 elem/cycle/partition).
  - Scores are computed transposed, S^T[k, q], 512 q at a time:
    matmul(lhsT=K^T_aug[65, 128], rhs=Q^T_aug[65, 512]) -> PSUM [128, 512].
    Work is a flat stream of (slot, kb, qh) units; exp groups of 3 units
    live in [128, 1536] PSUM tiles (3 banks, double-buffered = 6 banks).
  - P = exp(S'/8) is written as float16 (rel err ~5e-4, well inside the
    2e-2 gate).  AV is P-chunk-stationary: for each 128-query block,
    matmul(lhsT=P^T[128k, 128q], rhs=V_aug[128k, 65]) accumulates
    O[q, d] over key blocks in PSUM.  LdWeights is free on the PE, and the
    65 fp16 moving rows cost 65 cycles, so AV is ~2x cheaper than the
    moving-P orientation and the output lands Q-MAJOR.
  - V_aug has a ones column, so O[:, 64] is the softmax denominator.
    Normalization collapses to a [128,1]-per-partition scalar multiply:
    one DVE reciprocal on the 4 denominator columns + one tensor_tensor
    with a stride-0 broadcast AP.  No cross-partition broadcast of any
    kind (the baseline's PE/DMA reciprocal-row machinery is gone).
  - Outputs are written [q, d] per slot — the natural layout — so the
    host-side unshard is a pure batch reorder.

Engine budget per core (cost model): ScalarE ~42us (saturated), PE ~28us,
DVE ~8us, Pool: v-loads only.  Inputs fp16 (Q/K host-converted; scores err
~4e-3 absolute pre-softmax-scale), matmul accumulation in PSUM f32.
"""

import math
from contextlib import ExitStack

import numpy as np

import concourse.bass as bass
import concourse.bacc as bacc
import concourse.mybir as mybir
import concourse.tile as tile
from concourse.bass_utils import run_bass_kernel_spmd

F32 = mybir.dt.float32
F16 = mybir.dt.float16
EXP = mybir.ActivationFunctionType.Exp

B, L, D = 64, 1024, 64
N_CORES = 8
SLOTS = B // N_CORES  # batches per core
KB = 128              # key-block size (partition dim of S^T)
N_KB = L // KB
QH = 512              # q columns per matmul unit (moving-operand max)
NQH = L // QH
GROUP = 3             # 512-col score units per exp instruction (3 PSUM banks)
MASK_VAL = -60000.0   # fits fp16; exp(-60000/8) == 0
DV = D + 1            # V columns + ones (denominator) column


def build_kernel(counts):
    """counts[s] = number of 128-wide key blocks to process for slot s."""
    nc = bacc.Bacc()

    qt_d = nc.dram_tensor("qt", [SLOTS, DV, L], F16, kind="ExternalInput")
    kt_d = nc.dram_tensor("kt", [SLOTS, DV, L], F16, kind="ExternalInput")
    v_d = nc.dram_tensor("v", [SLOTS, KB, N_KB, DV], F16, kind="ExternalInput")
    out_d = nc.dram_tensor("out", [SLOTS, L, D], F32, kind="ExternalOutput")

    # Flat unit stream, qh-major inside each slot so the first exp only
    # needs half of qt; kb ascending keeps PSUM accumulation ordered.
    # Each unit carries its index within the slot (prefetch trigger points).
    units = []
    for s in range(SLOTS):
        for qh in range(NQH):
            for kb in range(counts[s]):
                units.append((s, kb, qh, qh * counts[s] + kb))
    # First groups are small so ScalarE starts as early as possible.
    lead = [1, 2]
    groups = []
    pos = 0
    for n in lead:
        if pos < len(units):
            groups.append(units[pos : pos + n])
            pos += n
    while pos < len(units):
        groups.append(units[pos : pos + GROUP])
        pos += GROUP
    n_groups = len(groups)

    with tile.TileContext(nc) as tc, ExitStack() as ctx:
        const_pool = ctx.enter_context(tc.tile_pool(name="const", bufs=1))
        qk_pool = ctx.enter_context(tc.tile_pool(name="qk", bufs=3))
        v_pool = ctx.enter_context(tc.tile_pool(name="v", bufs=3))
        p_pool = ctx.enter_context(tc.tile_pool(name="p", bufs=4))
        ep_pool = ctx.enter_context(tc.tile_pool(name="ep", bufs=4))
        out_pool = ctx.enter_context(tc.tile_pool(name="out", bufs=4))
        psum_s = ctx.enter_context(tc.tile_pool(name="psum_s", bufs=2, space="PSUM"))
        psum_o = ctx.enter_context(tc.tile_pool(name="psum_o", bufs=1, space="PSUM"))

        qt_tiles: dict[int, object] = {}
        kt_tiles: dict[int, object] = {}
        v_tiles: dict[int, object] = {}
        o_tiles: dict[tuple, object] = {}
        s_tiles: dict[int, object] = {}
        p_tiles: dict[int, object] = {}

        def load_kt(s):
            if s >= SLOTS or s in kt_tiles:
                return
            n_kb = counts[s]
            kt_t = qk_pool.tile([DV, L], F16, tag="kt", name="kt_t")
            if s == 0:
                # Piecewise so the first QK only waits on ~768B/partition.
                head = min(GROUP, n_kb) * KB
                nc.sync.dma_start(kt_t[:, :head], kt_d[0][:, :head])
                if n_kb * KB > head:
                    nc.sync.dma_start(
                        kt_t[:, head : n_kb * KB], kt_d[0][:, head : n_kb * KB]
                    )
            else:
                nc.sync.dma_start(kt_t[:, : n_kb * KB], kt_d[s][:, : n_kb * KB])
            kt_tiles[s] = kt_t

        def load_qt(s):
            if s >= SLOTS or s in qt_tiles:
                return
            qt_t = qk_pool.tile([DV, L], F16, tag="qt", name="qt_t")
            if s == 0:
                # Only the qh=0 half up front; the tail is deferred to group
                # 2 so its DGE config on the Activation sequencer never sits
                # in front of the first exps.
                nc.scalar.dma_start(qt_t[:, :QH], qt_d[0][:, :QH])
            else:
                nc.scalar.dma_start(qt_t[:], qt_d[s])
            qt_tiles[s] = qt_t

        def load_v(s):
            if s >= SLOTS or s in v_tiles:
                return
            n_kb = counts[s]
            v_t = v_pool.tile([KB, N_KB, DV], F16, name="v_t")
            nc.gpsimd.dma_start(v_t[:, :n_kb, :], v_d[s][:, :n_kb, :])
            v_tiles[s] = v_t

        def emit_qk(i):
            st = psum_s.tile([KB, GROUP * QH], F32, name="s_ps")
            s_tiles[i] = st
            for u, (s, kb, qh, islot) in enumerate(groups[i]):
                if islot == 0:
                    load_kt(s + 1)
                    load_v(s + 1)
                elif islot == 1:
                    load_qt(s + 1)
                nc.tensor.matmul(
                    st[:, u * QH : (u + 1) * QH],
                    kt_tiles[s][:, kb * KB : (kb + 1) * KB],
                    qt_tiles[s][:, qh * QH : (qh + 1) * QH],
                    start=True,
                    stop=True,
                )

        def emit_exp(i):
            w = len(groups[i]) * QH
            pt = p_pool.tile([KB, GROUP * QH], F16, name="p_t")
            p_tiles[i] = pt
            nc.scalar.activation(
                pt[:, :w], s_tiles.pop(i)[:, :w], EXP, scale=1.0 / math.sqrt(D)
            )

        def emit_av(i):
            pt = p_tiles.pop(i)
            for u, (s, kb, qh, islot) in enumerate(groups[i]):
                tag = "oA" if qh == 0 else "oB"
                if kb == 0:
                    o_tiles[(s, qh)] = psum_o.tile(
                        [KB, NQH * 2 * DV], F32, tag=tag, name=tag
                    )
                o = o_tiles[(s, qh)]
                last = kb == counts[s] - 1
                for j in range(4):
                    # start=True zeroes the WHOLE PSUM bank, so only the
                    # very first matmul into this o tile may use it; the
                    # other three q-block regions accumulate onto the
                    # zeroed bank.
                    nc.tensor.matmul(
                        o[:, j * DV : (j + 1) * DV],
                        pt[:, u * QH + j * KB : u * QH + (j + 1) * KB],
                        v_tiles[s][:, kb, :],
                        start=(kb == 0 and j == 0),
                        stop=last,
                        skip_group_check=True,
                    )
                if last:
                    emit_epilogue(s, qh)
                    if qh == NQH - 1:
                        qt_tiles.pop(s)
                        kt_tiles.pop(s)
                        v_tiles.pop(s)

        def emit_epilogue(s, half):
            o = o_tiles.pop((s, half))
            rec = ep_pool.tile([KB, 4], F32, name="rec")
            nc.vector.reciprocal(rec[:], o[:, D::DV])
            o3 = o[:].rearrange("p (a b) -> p a b", b=DV)[:, :, :D]
            rec3 = rec[:].rearrange("p (a b) -> p a b", b=1)
            final = s == SLOTS - 1 and half == NQH - 1
            osb = out_pool.tile([KB, 4 * D], F32, name="osb")
            nc.vector.tensor_tensor(
                osb[:].rearrange("p (a b) -> p a b", b=D),
                o3,
                rec3.broadcast_to([KB, 4, D]),
                op=mybir.AluOpType.mult,
            )
            dst = out_d[s][half * QH : (half + 1) * QH].rearrange(
                "(j p) d -> p j d", p=KB
            )
            nc.sync.dma_start(dst, osb[:].rearrange("p (a b) -> p a b", b=D))

        # Prologue: warm the exp table off the critical path, start loads.
        warm_in = const_pool.tile([1, 1], F32)
        warm_out = const_pool.tile([1, 1], F32)
        nc.gpsimd.memset(warm_in[:], 0.0)
        nc.scalar.activation(warm_out[:], warm_in[:], EXP)
        load_kt(0)
        load_qt(0)
        load_v(0)

        emit_qk(0)
        for i in range(n_groups):
            if i == 2:
                # Deferred tail of slot 0's qt (needed from its qh=1 sweep).
                nc.scalar.dma_start(qt_tiles[0][:, QH:], qt_d[0][:, QH:])
            if i + 1 < n_groups:
                emit_qk(i + 1)
            emit_exp(i)
            if i >= 1:
                emit_av(i - 1)
        emit_av(n_groups - 1)

    nc.finalize()
    return nc


_NC_CACHE: dict[tuple, object] = {}


def _prepare(queries, keys, values, valid_lens):
    queries = np.ascontiguousarray(queries, dtype=np.float32)
    keys = np.ascontiguousarray(keys, dtype=np.float32)
    values = np.ascontiguousarray(values, dtype=np.float32)
    valid_lens = np.asarray(valid_lens)
    assert queries.shape == (B, L, D), queries.shape
    vl = valid_lens.astype(np.int64)

    # Sort batches by valid_len descending; slot s on core c gets the batch
    # of rank s*8 + c.  counts[s] covers the rank-group max, so one SPMD
    # instruction stream fits all cores.
    order = np.argsort(-vl, kind="stable")
    counts = tuple(
        max(1, math.ceil(int(vl[order[s * N_CORES]]) / KB)) for s in range(SLOTS)
    )
    nc = _NC_CACHE.get(counts)
    if nc is None:
        nc = build_kernel(counts)
        _NC_CACHE[counts] = nc

    col = np.arange(L)
    in_maps = []
    for c in range(N_CORES):
        batch_idx = [int(order[s * N_CORES + c]) for s in range(SLOTS)]
        # Q^T / K^T with the extra contraction row: ones for Q, mask for K.
        qt = np.empty((SLOTS, DV, L), np.float16)
        qt[:, :D, :] = queries[batch_idx].transpose(0, 2, 1)
        qt[:, D, :] = 1.0
        kt = np.empty((SLOTS, DV, L), np.float16)
        kt[:, :D, :] = keys[batch_idx].transpose(0, 2, 1)
        kt[:, D, :] = (col[None, :] >= vl[batch_idx, None]) * np.float16(MASK_VAL)
        # V with ones column, pre-tiled [KB, N_KB, DV] per slot.
        v = np.empty((SLOTS, L, DV), np.float16)
        v[:, :, :D] = values[batch_idx]
        v[:, :, D] = 1.0
        v = np.ascontiguousarray(
            v.reshape(SLOTS, N_KB, KB, DV).transpose(0, 2, 1, 3)
        )
        in_maps.append({"qt": qt, "kt": kt, "v": v})
    return nc, in_maps, order


def _unshard(res, order):
    out = np.empty((B, L, D), dtype=np.float32)
    for c in range(N_CORES):
        o = res.results[c]["out"]  # [SLOTS, L, D]
        for s in range(SLOTS):
            out[int(order[s * N_CORES + c])] = o[s]
    return out


def kernel(queries, keys, values, valid_lens):
    nc, in_maps, order = _prepare(queries, keys, values, valid_lens)
    res = run_bass_kernel_spmd(nc, in_maps, core_ids=list(range(N_CORES)))
    return _unshard(res, order)


def trace_run(queries, keys, values, valid_lens):
    """Like kernel() but traced; returns BassKernelResults (for test.py)."""
    nc, in_maps, order = _prepare(queries, keys, values, valid_lens)
    res = run_bass_kernel_spmd(
        nc, in_maps, core_ids=list(range(N_CORES)), trace=True
    )
    res.full_output = _unshard(res, order)
    return res


# revision 30
# speedup vs baseline: 1.0691x; 1.0081x over previous
"""Masked dot-product attention (B=64, L=1024, D=64, fp32) on 8 NeuronCores.

Strategy (data-parallel over batch, per the sharding hint):
  - Batches are sorted by valid_len (descending) and dealt round-robin to the
    8 cores; slot s's key-block loop count is the max over its rank group, so
    one SPMD instruction stream fits all cores and fully-masked key blocks
    are never computed.
  - The sequence mask rides INSIDE the QK matmul as an extra contraction row:
    K^T is augmented with a mask row (0 / -60000 per key) and Q^T with a ones
    row, so S' = K^T.T @ Q^T + m[k] needs no per-block exp bias.  That makes
    the exp a pure elementwise op over PSUM columns, so one ScalarE
    instruction spans THREE 512-column score units regardless of which key
    block they belong to (amortizes the ~185ns per-instruction SBUF access
    latency; ScalarE is the bottleneck engine at ~1 elem/cycle/partition).
  - Scores are computed transposed, S^T[k, q], 512 q at a time:
    matmul(lhsT=K^T_aug[65, 128], rhs=Q^T_aug[65, 512]) -> PSUM [128, 512].
    Work is a flat stream of (slot, kb, qh) units; exp groups of 3 units
    live in [128, 1536] PSUM tiles (3 banks, double-buffered = 6 banks).
  - P = exp(S'/8) is written as float16 (rel err ~5e-4, well inside the
    2e-2 gate).  AV is P-chunk-stationary: for each 128-query block,
    matmul(lhsT=P^T[128k, 128q], rhs=V_aug[128k, 65]) accumulates
    O[q, d] over key blocks in PSUM.  LdWeights is free on the PE, and the
    65 fp16 moving rows cost 65 cycles, so AV is ~2x cheaper than the
    moving-P orientation and the output lands Q-MAJOR.
  - V_aug has a ones column, so O[:, 64] is the softmax denominator.
    Normalization collapses to a [128,1]-per-partition scalar multiply:
    one DVE reciprocal on the 4 denominator columns + one tensor_tensor
    with a stride-0 broadcast AP.  No cross-partition broadcast of any
    kind (the baseline's PE/DMA reciprocal-row machinery is gone).
  - Outputs are written [q, d] per slot — the natural layout — so the
    host-side unshard is a pure batch reorder.

Engine budget per core (cost model): ScalarE ~42us (saturated), PE ~28us,
DVE ~8us, Pool: v-loads only.  Inputs fp16 (Q/K host-converted; scores err
~4e-3 absolute pre-softmax-scale), matmul accumulation in PSUM f32.
"""

import math
from contextlib import ExitStack

import numpy as np

import concourse.bass as bass
import concourse.bacc as bacc
import concourse.mybir as mybir
import concourse.tile as tile
from concourse.bass_utils import run_bass_kernel_spmd

F32 = mybir.dt.float32
F16 = mybir.dt.float16
EXP = mybir.ActivationFunctionType.Exp

B, L, D = 64, 1024, 64
N_CORES = 8
SLOTS = B // N_CORES  # batches per core
KB = 128              # key-block size (partition dim of S^T)
N_KB = L // KB
QH = 512              # q columns per matmul unit (moving-operand max)
NQH = L // QH
GROUP = 3             # 512-col score units per exp instruction (3 PSUM banks)
MASK_VAL = -60000.0   # fits fp16; exp(-60000/8) == 0
DV = D + 1            # V columns + ones (denominator) column


def build_kernel(counts):
    """counts[s] = number of 128-wide key blocks to process for slot s."""
    nc = bacc.Bacc()

    qt_d = nc.dram_tensor("qt", [SLOTS, DV, L], F16, kind="ExternalInput")
    kt_d = nc.dram_tensor("kt", [SLOTS, DV, L], F16, kind="ExternalInput")
    v_d = nc.dram_tensor("v", [SLOTS, KB, N_KB, DV], F16, kind="ExternalInput")
    # Output layout is [slot, half, partition, qblock*D]: per-partition rows
    # are contiguous 1KB DMA descriptors (runs < 512B pay a 2x DMA latency
    # penalty); the host unshards with a cheap transpose.
    out_d = nc.dram_tensor("out", [SLOTS, NQH, KB, 4 * D], F32, kind="ExternalOutput")

    # Flat unit stream, qh-major inside each slot so the first exp only
    # needs half of qt; kb ascending keeps PSUM accumulation ordered.
    # Each unit carries its index within the slot (prefetch trigger points).
    units = []
    for s in range(SLOTS):
        for qh in range(NQH):
            for kb in range(counts[s]):
                units.append((s, kb, qh, qh * counts[s] + kb))
    # First groups are small so ScalarE starts as early as possible.
    lead = [1, 2]
    groups = []
    pos = 0
    for n in lead:
        if pos < len(units):
            groups.append(units[pos : pos + n])
            pos += n
    while pos < len(units):
        groups.append(units[pos : pos + GROUP])
        pos += GROUP
    n_groups = len(groups)

    with tile.TileContext(nc) as tc, ExitStack() as ctx:
        const_pool = ctx.enter_context(tc.tile_pool(name="const", bufs=1))
        qk_pool = ctx.enter_context(tc.tile_pool(name="qk", bufs=3))
        v_pool = ctx.enter_context(tc.tile_pool(name="v", bufs=3))
        p_pool = ctx.enter_context(tc.tile_pool(name="p", bufs=4))
        ep_pool = ctx.enter_context(tc.tile_pool(name="ep", bufs=4))
        out_pool = ctx.enter_context(tc.tile_pool(name="out", bufs=4))
        psum_s = ctx.enter_context(tc.tile_pool(name="psum_s", bufs=2, space="PSUM"))
        psum_o = ctx.enter_context(tc.tile_pool(name="psum_o", bufs=1, space="PSUM"))

        qt_tiles: dict[int, object] = {}
        kt_tiles: dict[int, object] = {}
        v_tiles: dict[int, object] = {}
        o_tiles: dict[tuple, object] = {}
        s_tiles: dict[int, object] = {}
        p_tiles: dict[int, object] = {}

        def load_kt(s):
            if s >= SLOTS or s in kt_tiles:
                return
            n_kb = counts[s]
            kt_t = qk_pool.tile([DV, L], F16, tag="kt", name="kt_t")
            if s == 0:
                # Piecewise; the head rides the scalar queue so it runs in
                # parallel with qt's half on the sync queue.
                head = min(GROUP, n_kb) * KB
                nc.sync.dma_start(kt_t[:, :head], kt_d[0][:, :head])
                if n_kb * KB > head:
                    nc.sync.dma_start(
                        kt_t[:, head : n_kb * KB], kt_d[0][:, head : n_kb * KB]
                    )
            else:
                nc.sync.dma_start(kt_t[:, : n_kb * KB], kt_d[s][:, : n_kb * KB])
            kt_tiles[s] = kt_t

        def load_qt(s):
            if s >= SLOTS or s in qt_tiles:
                return
            qt_t = qk_pool.tile([DV, L], F16, tag="qt", name="qt_t")
            if s == 0:
                # Only the qh=0 half up front; the tail is deferred to group
                # 2 so its DGE config never sits in front of the first exps.
                nc.scalar.dma_start(qt_t[:, :QH], qt_d[0][:, :QH])
            else:
                nc.scalar.dma_start(qt_t[:], qt_d[s])
            qt_tiles[s] = qt_t

        def load_v(s):
            if s >= SLOTS or s in v_tiles:
                return
            n_kb = counts[s]
            v_t = v_pool.tile([KB, N_KB, DV], F16, name="v_t")
            nc.gpsimd.dma_start(v_t[:, :n_kb, :], v_d[s][:, :n_kb, :])
            v_tiles[s] = v_t

        def emit_qk(i):
            st = psum_s.tile([KB, GROUP * QH], F32, name="s_ps")
            s_tiles[i] = st
            for u, (s, kb, qh, islot) in enumerate(groups[i]):
                # Prefetch the next slot.  Slot 0 defers its triggers a few
                # units so slot 1's transfers don't contend with slot 0's
                # critical first loads on the (serialized) DMA engines.
                kt_at, qt_at = (2, 3) if s == 0 else (0, 1)
                if islot == kt_at:
                    load_kt(s + 1)
                    load_v(s + 1)
                elif islot == qt_at:
                    load_qt(s + 1)
                if i == 0:
                    # Split the first matmul so the first exp starts ~0.5us
                    # earlier (the PE runs its first instruction at the low
                    # p-state).  Both halves share one PSUM bank and
                    # start=True zeroes the whole bank, so only the first
                    # half may use it.
                    for h in range(2):
                        c0, c1 = h * QH // 2, (h + 1) * QH // 2
                        nc.tensor.matmul(
                            st[:, c0:c1],
                            kt_tiles[s][:, kb * KB : (kb + 1) * KB],
                            qt_tiles[s][:, c0:c1],
                            start=(h == 0),
                            stop=True,
                            skip_group_check=True,
                        )
                else:
                    nc.tensor.matmul(
                        st[:, u * QH : (u + 1) * QH],
                        kt_tiles[s][:, kb * KB : (kb + 1) * KB],
                        qt_tiles[s][:, qh * QH : (qh + 1) * QH],
                        start=True,
                        stop=True,
                    )

        def emit_exp(i):
            w = len(groups[i]) * QH
            pt = p_pool.tile([KB, GROUP * QH], F16, name="p_t")
            p_tiles[i] = pt
            st = s_tiles.pop(i)
            ranges = [(0, w)]
            for c0, c1 in ranges:
                nc.scalar.activation(
                    pt[:, c0:c1], st[:, c0:c1], EXP, scale=1.0 / math.sqrt(D)
                )

        def emit_av(i):
            pt = p_tiles.pop(i)
            for u, (s, kb, qh, islot) in enumerate(groups[i]):
                tag = "oA" if qh == 0 else "oB"
                if kb == 0:
                    o_tiles[(s, qh)] = psum_o.tile(
                        [KB, NQH * 2 * DV], F32, tag=tag, name=tag
                    )
                o = o_tiles[(s, qh)]
                last = kb == counts[s] - 1
                for j in range(4):
                    # start=True zeroes the WHOLE PSUM bank, so only the
                    # very first matmul into this o tile may use it; the
                    # other three q-block regions accumulate onto the
                    # zeroed bank.
                    nc.tensor.matmul(
                        o[:, j * DV : (j + 1) * DV],
                        pt[:, u * QH + j * KB : u * QH + (j + 1) * KB],
                        v_tiles[s][:, kb, :],
                        start=(kb == 0 and j == 0),
                        stop=last,
                        skip_group_check=True,
                    )
                if last:
                    emit_epilogue(s, qh)
                    if qh == NQH - 1:
                        qt_tiles.pop(s)
                        kt_tiles.pop(s)
                        v_tiles.pop(s)

        def emit_epilogue(s, half):
            o = o_tiles.pop((s, half))
            rec = ep_pool.tile([KB, 4], F32, name="rec")
            nc.vector.reciprocal(rec[:], o[:, D::DV])
            o3 = o[:].rearrange("p (a b) -> p a b", b=DV)[:, :, :D]
            rec3 = rec[:].rearrange("p (a b) -> p a b", b=1)
            final = s == SLOTS - 1 and half == NQH - 1
            osb = out_pool.tile([KB, 4 * D], F32, name="osb")
            nc.vector.tensor_tensor(
                osb[:].rearrange("p (a b) -> p a b", b=D),
                o3,
                rec3.broadcast_to([KB, 4, D]),
                op=mybir.AluOpType.mult,
            )
            nc.sync.dma_start(out_d[s, half], osb[:])

        # Prologue: warm the exp table off the critical path, start loads.
        warm_in = const_pool.tile([1, 1], F32)
        warm_out = const_pool.tile([1, 1], F32)
        nc.gpsimd.memset(warm_in[:], 0.0)
        nc.scalar.activation(warm_out[:], warm_in[:], EXP)
        load_kt(0)
        load_qt(0)
        load_v(0)

        emit_qk(0)
        for i in range(n_groups):
            if i == 2:
                # Deferred tail of slot 0's qt (needed from its qh=1 sweep).
                nc.scalar.dma_start(qt_tiles[0][:, QH:], qt_d[0][:, QH:])
            if i + 1 < n_groups:
                emit_qk(i + 1)
            emit_exp(i)
            if i >= 1:
                emit_av(i - 1)
        emit_av(n_groups - 1)

    nc.finalize()
    return nc


_NC_CACHE: dict[tuple, object] = {}


def _prepare(queries, keys, values, valid_lens):
    queries = np.ascontiguousarray(queries, dtype=np.float32)
    keys = np.ascontiguousarray(keys, dtype=np.float32)
    values = np.ascontiguousarray(values, dtype=np.float32)
    valid_lens = np.asarray(valid_lens)
    assert queries.shape == (B, L, D), queries.shape
    vl = valid_lens.astype(np.int64)

    # Sort batches by valid_len descending; slot s on core c gets the batch
    # of rank s*8 + c.  counts[s] covers the rank-group max, so one SPMD
    # instruction stream fits all cores.
    order = np.argsort(-vl, kind="stable")
    counts = tuple(
        max(1, math.ceil(int(vl[order[s * N_CORES]]) / KB)) for s in range(SLOTS)
    )
    nc = _NC_CACHE.get(counts)
    if nc is None:
        nc = build_kernel(counts)
        _NC_CACHE[counts] = nc

    col = np.arange(L)
    in_maps = []
    for c in range(N_CORES):
        batch_idx = [int(order[s * N_CORES + c]) for s in range(SLOTS)]
        # Q^T / K^T with the extra contraction row: ones for Q, mask for K.
        qt = np.empty((SLOTS, DV, L), np.float16)
        qt[:, :D, :] = queries[batch_idx].transpose(0, 2, 1)
        qt[:, D, :] = 1.0
        kt = np.empty((SLOTS, DV, L), np.float16)
        kt[:, :D, :] = keys[batch_idx].transpose(0, 2, 1)
        kt[:, D, :] = (col[None, :] >= vl[batch_idx, None]) * np.float16(MASK_VAL)
        # V with ones column, pre-tiled [KB, N_KB, DV] per slot.
        v = np.empty((SLOTS, L, DV), np.float16)
        v[:, :, :D] = values[batch_idx]
        v[:, :, D] = 1.0
        v = np.ascontiguousarray(
            v.reshape(SLOTS, N_KB, KB, DV).transpose(0, 2, 1, 3)
        )
        in_maps.append({"qt": qt, "kt": kt, "v": v})
    return nc, in_maps, order


def _unshard(res, order):
    out = np.empty((B, L, D), dtype=np.float32)
    for c in range(N_CORES):
        o = res.results[c]["out"]  # [SLOTS, NQH, KB, 4*D]
        # [s, half, p, j*D] -> [s, half, j, p, D] -> [s, L, D]
        o = (
            o.reshape(SLOTS, NQH, KB, 4, D)
            .transpose(0, 1, 3, 2, 4)
            .reshape(SLOTS, L, D)
        )
        for s in range(SLOTS):
            out[int(order[s * N_CORES + c])] = o[s]
    return out


def kernel(queries, keys, values, valid_lens):
    nc, in_maps, order = _prepare(queries, keys, values, valid_lens)
    res = run_bass_kernel_spmd(nc, in_maps, core_ids=list(range(N_CORES)))
    return _unshard(res, order)


def trace_run(queries, keys, values, valid_lens):
    """Like kernel() but traced; returns BassKernelResults (for test.py)."""
    nc, in_maps, order = _prepare(queries, keys, values, valid_lens)
    res = run_bass_kernel_spmd(
        nc, in_maps, core_ids=list(range(N_CORES)), trace=True
    )
    res.full_output = _unshard(res, order)
    return res


# revision 40
# speedup vs baseline: 1.0716x; 1.0023x over previous
"""Masked dot-product attention (B=64, L=1024, D=64, fp32) on 8 NeuronCores.

Strategy (data-parallel over batch, per the sharding hint):
  - Batches are sorted by valid_len (descending) and dealt round-robin to the
    8 cores; slot s's key-block loop count is the max over its rank group, so
    one SPMD instruction stream fits all cores and fully-masked key blocks
    are never computed.
  - The sequence mask rides INSIDE the QK matmul as an extra contraction row:
    K^T is augmented with a mask row (0 / -60000 per key) and Q^T with a ones
    row, so S' = K^T.T @ Q^T + m[k] needs no per-block exp bias.  That makes
    the exp a pure elementwise op over PSUM columns, so one ScalarE
    instruction spans THREE 512-column score units regardless of which key
    block they belong to (amortizes the ~185ns per-instruction SBUF access
    latency; ScalarE is the bottleneck engine at ~1 elem/cycle/partition).
  - Scores are computed transposed, S^T[k, q], 512 q at a time:
    matmul(lhsT=K^T_aug[65, 128], rhs=Q^T_aug[65, 512]) -> PSUM [128, 512].
    Work is a flat stream of (slot, kb, qh) units; exp groups of 3 units
    live in [128, 1536] PSUM tiles (3 banks, double-buffered = 6 banks).
  - P = exp(S'/8) is written as float16 (rel err ~5e-4, well inside the
    2e-2 gate).  AV is P-chunk-stationary: for each 128-query block,
    matmul(lhsT=P^T[128k, 128q], rhs=V_aug[128k, 65]) accumulates
    O[q, d] over key blocks in PSUM.  LdWeights is free on the PE, and the
    65 fp16 moving rows cost 65 cycles, so AV is ~2x cheaper than the
    moving-P orientation and the output lands Q-MAJOR.
  - V_aug has a ones column, so O[:, 64] is the softmax denominator.
    Normalization collapses to a [128,1]-per-partition scalar multiply:
    one DVE reciprocal on the 4 denominator columns + one tensor_tensor
    with a stride-0 broadcast AP.  No cross-partition broadcast of any
    kind (the baseline's PE/DMA reciprocal-row machinery is gone).
  - Outputs are written q-major as [slot, half, partition, qblock*D] so the
    out-DMA descriptors are contiguous 1KB per-partition runs (sub-512B runs
    pay a 2x DMA latency penalty); the host unshard is a cheap transpose.

Engine budget per core (cost model, 51.9us total): ScalarE 42.2us busy and
gap-free from the first exp to the last — the kernel is ScalarE-bound, so
the only remaining overheads are the ~4.7us DMA-chain prologue before the
first exp and the ~4.9us epilogue/DMA tail after the last one.  PE ~28us,
DVE ~8us, Pool: v-loads only.  Inputs fp16 (Q/K host-converted; scores err
~4e-3 absolute pre-softmax-scale), matmul accumulation in PSUM f32.

Scheduling notes (in-order engine streams; emission order matters):
  - Pipeline per group i: emit QK(i+1), exp(i), AV(i-1).  PE runs ~0.5us
    of slack per group, so ScalarE never waits after the pipeline fills.
  - start=True on a matmul zeroes its ENTIRE PSUM bank, so only the first
    matmul into each o accumulator (and into the split first score unit)
    may use it; all other regions accumulate onto the zeroed bank.
  - Slot 0's prefetch triggers are deferred a few units and its qt tail to
    group 2, keeping the (serialized) DMA engines clear for the critical
    first loads; later slots prefetch at their first two units.
"""

import math
from contextlib import ExitStack

import numpy as np

import concourse.bass as bass
import concourse.bacc as bacc
import concourse.mybir as mybir
import concourse.tile as tile
from concourse.bass_utils import run_bass_kernel_spmd

F32 = mybir.dt.float32
F16 = mybir.dt.float16
EXP = mybir.ActivationFunctionType.Exp

B, L, D = 64, 1024, 64
N_CORES = 8
SLOTS = B // N_CORES  # batches per core
KB = 128              # key-block size (partition dim of S^T)
N_KB = L // KB
QH = 512              # q columns per matmul unit (moving-operand max)
NQH = L // QH
GROUP = 3             # 512-col score units per exp instruction (3 PSUM banks)
MASK_VAL = -60000.0   # fits fp16; exp(-60000/8) == 0
DV = D + 1            # V columns + ones (denominator) column


def build_kernel(counts):
    """counts[s] = number of 128-wide key blocks to process for slot s."""
    nc = bacc.Bacc()

    qt_d = nc.dram_tensor("qt", [SLOTS, DV, L], F16, kind="ExternalInput")
    kt_d = nc.dram_tensor("kt", [SLOTS, DV, L], F16, kind="ExternalInput")
    v_d = nc.dram_tensor("v", [SLOTS, KB, N_KB, DV], F16, kind="ExternalInput")
    # Output layout is [slot, half, partition, qblock*D]: per-partition rows
    # are contiguous 1KB DMA descriptors (runs < 512B pay a 2x DMA latency
    # penalty); the host unshards with a cheap transpose.
    out_d = nc.dram_tensor("out", [SLOTS, NQH, KB, 4 * D], F32, kind="ExternalOutput")

    # Flat unit stream, qh-major inside each slot so the first exp only
    # needs half of qt; kb ascending keeps PSUM accumulation ordered.
    # Each unit carries its index within the slot (prefetch trigger points).
    units = []
    for s in range(SLOTS):
        for qh in range(NQH):
            for kb in range(counts[s]):
                units.append((s, kb, qh, qh * counts[s] + kb))
    # First groups are small so ScalarE starts as early as possible.
    lead = [2, 2]
    groups = []
    pos = 0
    for n in lead:
        if pos < len(units):
            groups.append(units[pos : pos + n])
            pos += n
    while pos < len(units):
        groups.append(units[pos : pos + GROUP])
        pos += GROUP
    n_groups = len(groups)

    with tile.TileContext(nc) as tc, ExitStack() as ctx:
        const_pool = ctx.enter_context(tc.tile_pool(name="const", bufs=1))
        qk_pool = ctx.enter_context(tc.tile_pool(name="qk", bufs=3))
        v_pool = ctx.enter_context(tc.tile_pool(name="v", bufs=3))
        p_pool = ctx.enter_context(tc.tile_pool(name="p", bufs=4))
        ep_pool = ctx.enter_context(tc.tile_pool(name="ep", bufs=4))
        out_pool = ctx.enter_context(tc.tile_pool(name="out", bufs=4))
        psum_s = ctx.enter_context(tc.tile_pool(name="psum_s", bufs=2, space="PSUM"))
        psum_o = ctx.enter_context(tc.tile_pool(name="psum_o", bufs=1, space="PSUM"))

        qt_tiles: dict[int, object] = {}
        kt_tiles: dict[int, object] = {}
        v_tiles: dict[int, object] = {}
        o_tiles: dict[tuple, object] = {}
        s_tiles: dict[int, object] = {}
        p_tiles: dict[int, object] = {}

        def load_kt(s):
            if s >= SLOTS or s in kt_tiles:
                return
            n_kb = counts[s]
            kt_t = qk_pool.tile([DV, L], F16, tag="kt", name="kt_t")
            if s == 0:
                # Piecewise; the head rides the scalar queue so it runs in
                # parallel with qt's half on the sync queue.
                head = min(GROUP, n_kb) * KB
                nc.sync.dma_start(kt_t[:, :head], kt_d[0][:, :head])
                if n_kb * KB > head:
                    nc.sync.dma_start(
                        kt_t[:, head : n_kb * KB], kt_d[0][:, head : n_kb * KB]
                    )
            else:
                nc.sync.dma_start(kt_t[:, : n_kb * KB], kt_d[s][:, : n_kb * KB])
            kt_tiles[s] = kt_t

        def load_qt(s):
            if s >= SLOTS or s in qt_tiles:
                return
            qt_t = qk_pool.tile([DV, L], F16, tag="qt", name="qt_t")
            if s == 0:
                # Only the qh=0 half up front; the tail is deferred to group
                # 2 so its DGE config never sits in front of the first exps.
                nc.scalar.dma_start(qt_t[:, :QH], qt_d[0][:, :QH])
            else:
                nc.scalar.dma_start(qt_t[:], qt_d[s])
            qt_tiles[s] = qt_t

        def load_v(s):
            if s >= SLOTS or s in v_tiles:
                return
            n_kb = counts[s]
            v_t = v_pool.tile([KB, N_KB, DV], F16, name="v_t")
            nc.gpsimd.dma_start(v_t[:, :n_kb, :], v_d[s][:, :n_kb, :])
            v_tiles[s] = v_t

        def emit_qk(i):
            st = psum_s.tile([KB, GROUP * QH], F32, name="s_ps")
            s_tiles[i] = st
            for u, (s, kb, qh, islot) in enumerate(groups[i]):
                # Prefetch the next slot.  Slot 0 defers its triggers a few
                # units so slot 1's transfers don't contend with slot 0's
                # critical first loads on the (serialized) DMA engines.
                kt_at, qt_at = (2, 3) if s == 0 else (0, 1)
                if islot == kt_at:
                    load_kt(s + 1)
                    load_v(s + 1)
                elif islot == qt_at:
                    load_qt(s + 1)
                if i == 0 and u == 0:
                    # Split the very first matmul so the first exp starts
                    # ~0.5us earlier (the PE runs its first instruction at
                    # the low p-state).  Both halves share one PSUM bank and
                    # start=True zeroes the whole bank, so only the first
                    # half may use it.
                    for h in range(2):
                        c0, c1 = h * QH // 2, (h + 1) * QH // 2
                        nc.tensor.matmul(
                            st[:, c0:c1],
                            kt_tiles[s][:, kb * KB : (kb + 1) * KB],
                            qt_tiles[s][:, c0:c1],
                            start=(h == 0),
                            stop=True,
                            skip_group_check=True,
                        )
                else:
                    nc.tensor.matmul(
                        st[:, u * QH : (u + 1) * QH],
                        kt_tiles[s][:, kb * KB : (kb + 1) * KB],
                        qt_tiles[s][:, qh * QH : (qh + 1) * QH],
                        start=True,
                        stop=True,
                    )

        def emit_exp(i):
            w = len(groups[i]) * QH
            pt = p_pool.tile([KB, GROUP * QH], F16, name="p_t")
            p_tiles[i] = pt
            st = s_tiles.pop(i)
            ranges = [(0, w)]
            for c0, c1 in ranges:
                nc.scalar.activation(
                    pt[:, c0:c1], st[:, c0:c1], EXP, scale=1.0 / math.sqrt(D)
                )

        def emit_av(i):
            pt = p_tiles.pop(i)
            for u, (s, kb, qh, islot) in enumerate(groups[i]):
                tag = "oA" if qh == 0 else "oB"
                if kb == 0:
                    o_tiles[(s, qh)] = psum_o.tile(
                        [KB, NQH * 2 * DV], F32, tag=tag, name=tag
                    )
                o = o_tiles[(s, qh)]
                last = kb == counts[s] - 1
                for j in range(4):
                    # start=True zeroes the WHOLE PSUM bank, so only the
                    # very first matmul into this o tile may use it; the
                    # other three q-block regions accumulate onto the
                    # zeroed bank.
                    nc.tensor.matmul(
                        o[:, j * DV : (j + 1) * DV],
                        pt[:, u * QH + j * KB : u * QH + (j + 1) * KB],
                        v_tiles[s][:, kb, :],
                        start=(kb == 0 and j == 0),
                        stop=last,
                        skip_group_check=True,
                    )
                if last:
                    emit_epilogue(s, qh)
                    if qh == NQH - 1:
                        qt_tiles.pop(s)
                        kt_tiles.pop(s)
                        v_tiles.pop(s)

        def emit_epilogue(s, half):
            o = o_tiles.pop((s, half))
            rec = ep_pool.tile([KB, 4], F32, name="rec")
            nc.vector.reciprocal(rec[:], o[:, D::DV])
            o3 = o[:].rearrange("p (a b) -> p a b", b=DV)[:, :, :D]
            rec3 = rec[:].rearrange("p (a b) -> p a b", b=1)
            final = s == SLOTS - 1 and half == NQH - 1
            osb = out_pool.tile([KB, 4 * D], F32, name="osb")
            nc.vector.tensor_tensor(
                osb[:].rearrange("p (a b) -> p a b", b=D),
                o3,
                rec3.broadcast_to([KB, 4, D]),
                op=mybir.AluOpType.mult,
            )
            nc.sync.dma_start(out_d[s, half], osb[:])

        # Prologue: warm the exp table off the critical path, start loads.
        warm_in = const_pool.tile([1, 1], F32)
        warm_out = const_pool.tile([1, 1], F32)
        nc.gpsimd.memset(warm_in[:], 0.0)
        nc.scalar.activation(warm_out[:], warm_in[:], EXP)
        load_kt(0)
        load_qt(0)
        load_v(0)

        emit_qk(0)
        for i in range(n_groups):
            if i == 2:
                # Deferred tail of slot 0's qt (needed from its qh=1 sweep).
                nc.scalar.dma_start(qt_tiles[0][:, QH:], qt_d[0][:, QH:])
            if i + 1 < n_groups:
                emit_qk(i + 1)
            emit_exp(i)
            if i >= 1:
                emit_av(i - 1)
        emit_av(n_groups - 1)

    nc.finalize()
    return nc


_NC_CACHE: dict[tuple, object] = {}


def _prepare(queries, keys, values, valid_lens):
    queries = np.ascontiguousarray(queries, dtype=np.float32)
    keys = np.ascontiguousarray(keys, dtype=np.float32)
    values = np.ascontiguousarray(values, dtype=np.float32)
    valid_lens = np.asarray(valid_lens)
    assert queries.shape == (B, L, D), queries.shape
    vl = valid_lens.astype(np.int64)

    # Sort batches by valid_len descending; slot s on core c gets the batch
    # of rank s*8 + c.  counts[s] covers the rank-group max, so one SPMD
    # instruction stream fits all cores.
    order = np.argsort(-vl, kind="stable")
    counts = tuple(
        max(1, math.ceil(int(vl[order[s * N_CORES]]) / KB)) for s in range(SLOTS)
    )
    nc = _NC_CACHE.get(counts)
    if nc is None:
        nc = build_kernel(counts)
        _NC_CACHE[counts] = nc

    col = np.arange(L)
    in_maps = []
    for c in range(N_CORES):
        batch_idx = [int(order[s * N_CORES + c]) for s in range(SLOTS)]
        # Q^T / K^T with the extra contraction row: ones for Q, mask for K.
        qt = np.empty((SLOTS, DV, L), np.float16)
        qt[:, :D, :] = queries[batch_idx].transpose(0, 2, 1)
        qt[:, D, :] = 1.0
        kt = np.empty((SLOTS, DV, L), np.float16)
        kt[:, :D, :] = keys[batch_idx].transpose(0, 2, 1)
        kt[:, D, :] = (col[None, :] >= vl[batch_idx, None]) * np.float16(MASK_VAL)
        # V with ones column, pre-tiled [KB, N_KB, DV] per slot.
        v = np.empty((SLOTS, L, DV), np.float16)
        v[:, :, :D] = values[batch_idx]
        v[:, :, D] = 1.0
        v = np.ascontiguousarray(
            v.reshape(SLOTS, N_KB, KB, DV).transpose(0, 2, 1, 3)
        )
        in_maps.append({"qt": qt, "kt": kt, "v": v})
    return nc, in_maps, order


def _unshard(res, order):
    out = np.empty((B, L, D), dtype=np.float32)
    for c in range(N_CORES):
        o = res.results[c]["out"]  # [SLOTS, NQH, KB, 4*D]
        # [s, half, p, j*D] -> [s, half, j, p, D] -> [s, L, D]
        o = (
            o.reshape(SLOTS, NQH, KB, 4, D)
            .transpose(0, 1, 3, 2, 4)
            .reshape(SLOTS, L, D)
        )
        for s in range(SLOTS):
            out[int(order[s * N_CORES + c])] = o[s]
    return out


def kernel(queries, keys, values, valid_lens):
    nc, in_maps, order = _prepare(queries, keys, values, valid_lens)
    res = run_bass_kernel_spmd(nc, in_maps, core_ids=list(range(N_CORES)))
    return _unshard(res, order)


def trace_run(queries, keys, values, valid_lens):
    """Like kernel() but traced; returns BassKernelResults (for test.py)."""
    nc, in_maps, order = _prepare(queries, keys, values, valid_lens)
    res = run_bass_kernel_spmd(
        nc, in_maps, core_ids=list(range(N_CORES)), trace=True
    )
    res.full_output = _unshard(res, order)
    return res
